# revision 4
# baseline (speedup 1.0000x reference)
"""DGMC-style graph matching network on 8 Trainium2 NeuronCores.

Reference math:
  psi(x) = relu(((I + A) @ x) @ W)   with A = dense ea-weighted adjacency
  h_s/h_t = psi(x_s/x_t, W1);  S_hat0[b] = h_s[b] @ h_t[b]^T
  10 steps: S = softmax(S_hat); r_t = S^T r_i; o_s/o_t = psi(r_i / r_t, W2)
            P_s = o_s@mw1 + mb1; P_t = o_t@mw1
            S_hat[s,t] += sum_c relu(P_s[s,c] - P_t[t,c]) * mw2[c]   (+mb2 dropped:
            a uniform logit shift cancels in every softmax downstream)
  outputs (softmax(S_hat0), softmax(S_hat_final))

Sharding: each of the 8 cores owns 256 consecutive global s-rows (2 cores
per graph). Edges are global (randint over all 2048 nodes), so o_t needs the
full r_t every step: each core computes its partial (S^T r) over its own
s-rows, a 64KB AllGather shares the 8 partials, and pair-sums rebuild r_t.
The adjacency (+I) is densified on the host; (I+A)@x runs as PE matmuls
against SBUF-resident A^T row-blocks. S_hat lives in PSUM for the whole
kernel and the relu'd pair-channel terms are reduced straight onto it with
tile_position matmuls (4 s-rows x 32 channels per 128-partition tile).
"""

import sys

import numpy as np

if "/opt/trn_rl_repo" not in sys.path:
    sys.path.insert(0, "/opt/trn_rl_repo")

B, NS, NT = 4, 512, 512
D_IN, C1, R, C2 = 128, 64, 32, 32
STEPS = 10
N = B * NS            # 2048 nodes per side
NCORES = 8
SROWS = N // NCORES   # 256 s-rows per core
KCH = N // 128        # 16 contraction chunks

_CACHE = {}


def _build_nc(steps=STEPS):
    import concourse.bacc as bacc
    import concourse.mybir as mybir
    import concourse.tile as tile

    DT = mybir.dt.float32
    AX = mybir.AxisListType
    OP = mybir.AluOpType
    AF = mybir.ActivationFunctionType

    nc = bacc.Bacc(None, target_bir_lowering=False, num_devices=NCORES)

    AsT_d = nc.declare_dram_parameter("AsT", [KCH, 128, SROWS], DT, isOutput=False)
    AtT_d = nc.declare_dram_parameter("AtT", [KCH, 128, NT], DT, isOutput=False)
    xsch_d = nc.declare_dram_parameter("xsch", [KCH, 128, D_IN], DT, isOutput=False)
    xtch_d = nc.declare_dram_parameter("xtch", [KCH, 128, D_IN], DT, isOutput=False)
    rch_d = nc.declare_dram_parameter("rch", [steps, KCH, 128, R], DT, isOutput=False)
    rown_d = nc.declare_dram_parameter("rown", [steps, 2, 128, R], DT, isOutput=False)
    w1_d = nc.declare_dram_parameter("w1", [D_IN, C1], DT, isOutput=False)
    w2_d = nc.declare_dram_parameter("w2", [R, C2], DT, isOutput=False)
    mw1_d = nc.declare_dram_parameter("mw1", [C2, C2], DT, isOutput=False)
    mb1_d = nc.declare_dram_parameter("mb1", [C2, 1], DT, isOutput=False)
    wblk_d = nc.declare_dram_parameter("wblk", [128, 8, 32], DT, isOutput=False)
    ident_d = nc.declare_dram_parameter("ident", [128, 128], DT, isOutput=False)
    s0_d = nc.declare_dram_parameter("S0", [SROWS, NT], DT, isOutput=True)
    sl_d = nc.declare_dram_parameter("SL", [SROWS, NT], DT, isOutput=True)

    with tile.TileContext(nc) as tc:
        with (
            tc.tile_pool(name="const", bufs=1) as cpool,
            tc.tile_pool(name="work", bufs=2) as wpool,
            tc.tile_pool(name="tg", bufs=6) as tgpool,
            tc.tile_pool(name="dram", bufs=2, space="DRAM") as dpool,
            tc.tile_pool(name="ps_shat", bufs=1, space="PSUM") as pshat,
            tc.tile_pool(name="ps_tmp", bufs=2, space="PSUM") as ptmp,
            tc.tile_pool(name="ps_tr", bufs=2, space="PSUM") as ptr,
            tc.tile_pool(name="ps_sch", bufs=2, space="PSUM") as psch,
        ):
            # ---- load constants ----
            a_sT = cpool.tile([128, KCH, SROWS], DT)
            nc.sync.dma_start(a_sT[:], AsT_d.rearrange("j p s -> p j s"))
            a_tT = cpool.tile([128, KCH, NT], DT)
            nc.sync.dma_start(a_tT[:], AtT_d.rearrange("j p s -> p j s"))
            xsch = cpool.tile([128, KCH, D_IN], DT)
            nc.sync.dma_start(xsch[:], xsch_d.rearrange("j p s -> p j s"))
            xtch = cpool.tile([128, KCH, D_IN], DT)
            nc.sync.dma_start(xtch[:], xtch_d.rearrange("j p s -> p j s"))
            rch = cpool.tile([128, steps, KCH, R], DT)
            for i in range(steps):
                nc.sync.dma_start(rch[:, i, :, :], rch_d[i].rearrange("j p c -> p j c"))
            rown = cpool.tile([128, steps, 2, R], DT)
            for i in range(steps):
                nc.sync.dma_start(rown[:, i, :, :], rown_d[i].rearrange("q p c -> p q c"))
            w1 = cpool.tile([128, C1], DT)
            nc.sync.dma_start(w1[:], w1_d[:])
            w2 = cpool.tile([R, C2], DT)
            nc.sync.dma_start(w2[:], w2_d[:])
            mw1 = cpool.tile([C2, C2], DT)
            nc.sync.dma_start(mw1[:], mw1_d[:])
            mb1 = cpool.tile([C2, 1], DT)
            nc.sync.dma_start(mb1[:], mb1_d[:])
            wblk = cpool.tile([128, 8, 32], DT)
            nc.sync.dma_start(wblk[:], wblk_d.rearrange("p v m -> p v m"))
            ident = cpool.tile([128, 128], DT)
            nc.sync.dma_start(ident[:], ident_d[:])

            # S_hat, PSUM-resident for the whole kernel (one bank per s-tile)
            shat = [pshat.tile([128, NT], DT, tag=f"shat{q}", name=f"shat{q}")
                    for q in range(2)]

            # ---- phase 1: h_s/h_t and S_hat0 ----
            usT_ps = ptmp.tile([128, SROWS], DT, tag="tmp", name="usT_ps")
            for j in range(KCH):
                nc.tensor.matmul(usT_ps[:], xsch[:, j, :], a_sT[:, j, :],
                                 start=(j == 0), stop=(j == KCH - 1))
            usT = wpool.tile([128, SROWS], DT, name="usT")
            nc.scalar.copy(usT[:], usT_ps[:])
            hsT_ps = ptmp.tile([C1, SROWS], DT, tag="tmp", name="hsT_ps")
            nc.tensor.matmul(hsT_ps[:], w1[:], usT[:], start=True, stop=True)
            hsT = wpool.tile([C1, SROWS], DT, name="hsT")
            nc.scalar.activation(hsT[:], hsT_ps[:], AF.Relu)

            utT_ps = ptmp.tile([128, NT], DT, tag="tmp", name="utT_ps")
            for j in range(KCH):
                nc.tensor.matmul(utT_ps[:], xtch[:, j, :], a_tT[:, j, :],
                                 start=(j == 0), stop=(j == KCH - 1))
            utT = wpool.tile([128, NT], DT, name="utT")
            nc.scalar.copy(utT[:], utT_ps[:])
            htT_ps = ptmp.tile([C1, NT], DT, tag="tmp", name="htT_ps")
            nc.tensor.matmul(htT_ps[:], w1[:], utT[:], start=True, stop=True)
            htT = wpool.tile([C1, NT], DT, name="htT")
            nc.scalar.activation(htT[:], htT_ps[:], AF.Relu)

            for q in range(2):
                nc.tensor.matmul(shat[q][:], hsT[:, q * 128:(q + 1) * 128], htT[:],
                                 start=True, stop=False, skip_group_check=True)

            # ---- step loop ----
            for i in range(steps):
                s_exp = []
                rinvs = []
                for q in range(2):
                    nmax = wpool.tile([128, 1], DT, tag="nmax", name="nmax")
                    nc.vector.tensor_reduce(nmax[:], shat[q][:], axis=AX.X,
                                            op=OP.max, negate=True)
                    se = wpool.tile([128, NT], DT, tag="sexp", name="se")
                    rsum = wpool.tile([128, 1], DT, tag="rsum", name="rsum")
                    nc.scalar.activation(se[:], shat[q][:], AF.Exp,
                                         bias=nmax[:, 0:1], accum_out=rsum[:, 0:1])
                    rinv = wpool.tile([128, 1], DT, tag="rinv", name="rinv")
                    nc.vector.reciprocal(rinv[:], rsum[:])
                    s_exp.append(se)
                    rinvs.append(rinv)

                if i == 0:
                    for q in range(2):
                        s0st = wpool.tile([128, NT], DT, tag="sst", name="s0st")
                        nc.vector.tensor_scalar(s0st[:], s_exp[q][:],
                                                rinvs[q][:, 0:1], None, op0=OP.mult)
                        nc.sync.dma_start(s0_d[q * 128:(q + 1) * 128, :], s0st[:])

                # partial (S^T r)^T over own s-rows -> [R, NT] (softmax 1/sum
                # folded into the moving operand)
                partial_ps = ptmp.tile([R, NT], DT, tag="tmp", name="partial_ps")
                for q in range(2):
                    rsc = wpool.tile([128, R], DT, tag="rsc", name="rsc")
                    nc.vector.tensor_scalar(rsc[:], rown[:, i, q, :],
                                            rinvs[q][:, 0:1], None, op0=OP.mult)
                    nc.tensor.matmul(partial_ps[:], rsc[:], s_exp[q][:],
                                     start=(q == 0), stop=(q == 1))
                partial_sb = wpool.tile([R, NT], DT, name="partial_sb")
                nc.scalar.copy(partial_sb[:], partial_ps[:])

                # transpose own partial to [t, ch] rows, ship through AllGather
                pN_ps = ptr.tile([128, 128], DT, tag="tr", name="pN_ps")
                for k in range(4):
                    nc.tensor.matmul(pN_ps[:, k * 32:(k + 1) * 32],
                                     partial_sb[:, k * 128:(k + 1) * 128],
                                     ident[0:32, 0:32], is_transpose=True,
                                     start=True, stop=True, skip_group_check=True)
                pN_sb = wpool.tile([128, 128], DT, name="pN_sb")
                nc.vector.tensor_copy(pN_sb[:], pN_ps[:])

                cc_in = dpool.tile([NT, R], DT, name="cc_in")
                nc.sync.dma_start(cc_in.rearrange("(k p) c -> p k c", p=128), pN_sb.rearrange("p (k c) -> p k c", k=4))
                cc_out = dpool.tile([NCORES, NT, R], DT, name="cc_out")
                nc.gpsimd.collective_compute(
                    "AllGather", OP.bypass,
                    replica_groups=[list(range(NCORES))],
                    ins=[cc_in[:]], outs=[cc_out[:]],
                )

                gath_ev = wpool.tile([128, KCH // 2, 2, R], DT, name="gath_ev")
                gath_od = wpool.tile([128, KCH // 2, 2, R], DT, name="gath_od")
                for g in range(B):
                    nc.sync.dma_start(
                        gath_ev[:, 2 * g:2 * g + 2, :, :],
                        cc_out[2 * g].rearrange("(k p) c -> p k c", p=128).rearrange("p (a b) c -> p a b c", a=2))
                    nc.sync.dma_start(
                        gath_od[:, 2 * g:2 * g + 2, :, :],
                        cc_out[2 * g + 1].rearrange("(k p) c -> p k c", p=128).rearrange("p (a b) c -> p a b c", a=2))
                rt = wpool.tile([128, KCH, R], DT, name="rt")
                nc.vector.tensor_tensor(rt.rearrange("p j c -> p (j c)"),
                                        gath_ev.rearrange("p a b c -> p (a b c)"),
                                        gath_od.rearrange("p a b c -> p (a b c)"),
                                        op=OP.add)

                # u_t^T = ((I + A_t) @ r_t)^T for own graph's 512 t-rows
                ut_ps = ptmp.tile([R, NT], DT, tag="tmp", name="ut_ps")
                for j in range(KCH):
                    nc.tensor.matmul(ut_ps[:], rt[:, j, :], a_tT[:, j, :],
                                     start=(j == 0), stop=(j == KCH - 1))
                ut_sb = wpool.tile([R, NT], DT, name="ut_sb")
                nc.scalar.copy(ut_sb[:], ut_ps[:])
                ot_ps = ptmp.tile([C2, NT], DT, tag="tmp", name="ot_ps")
                nc.tensor.matmul(ot_ps[:], w2[:], ut_sb[:], start=True, stop=True)
                ot_sb = wpool.tile([C2, NT], DT, name="ot_sb")
                nc.scalar.activation(ot_sb[:], ot_ps[:], AF.Relu)
                pt_ps = ptmp.tile([C2, NT], DT, tag="tmp", name="pt_ps")
                nc.tensor.matmul(pt_ps[:], mw1[:], ot_sb[:], start=True, stop=True)
                # Bneg = -P_t^T replicated over the 4 s-slots of each partition block
                bneg = wpool.tile([128, NT], DT, name="bneg")
                for j in range(4):
                    nc.vector.tensor_scalar(bneg[32 * j:32 * (j + 1), :], pt_ps[:],
                                            -1.0, None, op0=OP.mult)

                # P_s chain for this step (independent of the collective; fills
                # the AllGather bubble): m_s^T = ((I+A_s) r_i)^T over own s-rows
                ms_ps = psch.tile([R, SROWS], DT, tag="sch", name="ms_ps")
                for j in range(KCH):
                    nc.tensor.matmul(ms_ps[:], rch[:, i, j, :], a_sT[:, j, :],
                                     start=(j == 0), stop=(j == KCH - 1))
                ms_sb = wpool.tile([R, SROWS], DT, name="ms_sb")
                nc.scalar.copy(ms_sb[:], ms_ps[:])
                os_ps = psch.tile([C2, SROWS], DT, tag="sch", name="os_ps")
                nc.tensor.matmul(os_ps[:], w2[:], ms_sb[:], start=True, stop=True)
                os_sb = wpool.tile([C2, SROWS], DT, name="os_sb")
                nc.scalar.activation(os_sb[:], os_ps[:], AF.Relu)
                ps_ps = psch.tile([C2, SROWS], DT, tag="sch", name="ps_ps")
                nc.tensor.matmul(ps_ps[:], mw1[:], os_sb[:], start=True, stop=True)
                # Avec[32j+c, 32q+g] = P_s[s=128q+4g+j, c] + mb1[c]
                avec = wpool.tile([128, 64], DT, name="avec")
                for q in range(2):
                    view = ps_ps[:, q * 128:(q + 1) * 128].rearrange(
                        "p (g j) -> p g j", j=4)
                    for j in range(4):
                        nc.vector.tensor_scalar(avec[32 * j:32 * (j + 1),
                                                     q * 32:(q + 1) * 32],
                                                view[:, :, j], mb1[:, 0:1],
                                                None, op0=OP.add)

                # update: S_hat[q][4g:4g+4, :] += sum_c relu(P_s - P_t) * mw2[c]
                for q in range(2):
                    for g in range(32):
                        col = q * 32 + g
                        tg = tgpool.tile([128, NT], DT, tag="tg", name="tg")
                        if g % 3 == 2:
                            nc.scalar.activation(tg[:], bneg[:], AF.Relu,
                                                 bias=avec[:, col:col + 1])
                        else:
                            nc.vector.tensor_scalar(tg[:], bneg[:],
                                                    avec[:, col:col + 1], 0.0,
                                                    op0=OP.add, op1=OP.max)
                        k = g // 8
                        nc.tensor.matmul(shat[q][32 * k:32 * (k + 1), :],
                                         wblk[:, g % 8, :], tg[:],
                                         start=False, stop=False,
                                         skip_group_check=True,
                                         tile_position=(0, 32 * k))

            # ---- final softmax -> S_L ----
            for q in range(2):
                nmaxf = wpool.tile([128, 1], DT, tag="nmax", name="nmaxf")
                nc.vector.tensor_reduce(nmaxf[:], shat[q][:], axis=AX.X,
                                        op=OP.max, negate=True)
                sef = wpool.tile([128, NT], DT, tag="sexp", name="sef")
                rsumf = wpool.tile([128, 1], DT, tag="rsum", name="rsumf")
                nc.scalar.activation(sef[:], shat[q][:], AF.Exp,
                                     bias=nmaxf[:, 0:1], accum_out=rsumf[:, 0:1])
                rinvf = wpool.tile([128, 1], DT, tag="rinv", name="rinvf")
                nc.vector.reciprocal(rinvf[:], rsumf[:])
                slst = wpool.tile([128, NT], DT, tag="sst", name="slst")
                nc.vector.tensor_scalar(slst[:], sef[:], rinvf[:, 0:1],
                                        None, op0=OP.mult)
                nc.sync.dma_start(sl_d[q * 128:(q + 1) * 128, :], slst[:])

    nc.compile()
    return nc


def _host_prep(inputs, steps=STEPS):
    x_s = np.asarray(inputs["x_s"], np.float32)
    x_t = np.asarray(inputs["x_t"], np.float32)
    ei_s = np.asarray(inputs["edge_index_s"])
    ei_t = np.asarray(inputs["edge_index_t"])
    ea_s = np.asarray(inputs["edge_attr_s"], np.float32)
    ea_t = np.asarray(inputs["edge_attr_t"], np.float32)
    W1 = np.asarray(inputs["W1"], np.float32)
    W2 = np.asarray(inputs["W2"], np.float32)
    mw1 = np.asarray(inputs["mw1"], np.float32)
    mb1 = np.asarray(inputs["mb1"], np.float32)
    mw2 = np.asarray(inputs["mw2"], np.float32)
    r = np.asarray(inputs["r"], np.float32).reshape(-1, N, R)[:steps]

    A_s = np.zeros((N, N), np.float32)
    np.add.at(A_s, (ei_s[1], ei_s[0]), ea_s)
    A_s[np.arange(N), np.arange(N)] += 1.0
    A_t = np.zeros((N, N), np.float32)
    np.add.at(A_t, (ei_t[1], ei_t[0]), ea_t)
    A_t[np.arange(N), np.arange(N)] += 1.0

    rch = np.ascontiguousarray(r.reshape(steps, KCH, 128, R))
    xsch = np.ascontiguousarray(x_s.reshape(KCH, 128, D_IN))
    xtch = np.ascontiguousarray(x_t.reshape(KCH, 128, D_IN))
    wblk = np.zeros((128, 8, 32), np.float32)
    for v in range(8):
        for j in range(4):
            wblk[32 * j:32 * (j + 1), v, 4 * v + j] = mw2[:, 0]
    ident = np.eye(128, dtype=np.float32)
    mb1c = np.ascontiguousarray(mb1.reshape(C2, 1))

    in_maps = []
    for c in range(NCORES):
        rows = slice(SROWS * c, SROWS * (c + 1))
        trows = slice(NT * (c // 2), NT * (c // 2 + 1))
        AsT = np.ascontiguousarray(A_s[rows, :].T).reshape(KCH, 128, SROWS)
        AtT = np.ascontiguousarray(A_t[trows, :].T).reshape(KCH, 128, NT)
        rown = np.ascontiguousarray(
            r[:, SROWS * c:SROWS * (c + 1), :].reshape(steps, 2, 128, R))
        in_maps.append({
            "AsT": AsT, "AtT": AtT, "xsch": xsch, "xtch": xtch,
            "rch": rch, "rown": rown, "w1": W1, "w2": W2, "mw1": mw1,
            "mb1": mb1c, "wblk": wblk, "ident": ident,
        })
    return in_maps


def kernel(**inputs):
    from concourse.bass_utils import run_bass_kernel_spmd

    if "nc" not in _CACHE:
        _CACHE["nc"] = _build_nc(STEPS)
    nc = _CACHE["nc"]

    in_maps = _host_prep(inputs, STEPS)
    res = run_bass_kernel_spmd(nc, in_maps, core_ids=list(range(NCORES)))
    outs = res.results
    S0 = np.concatenate([outs[c]["S0"] for c in range(NCORES)], axis=0)
    SL = np.concatenate([outs[c]["SL"] for c in range(NCORES)], axis=0)
    return (S0.reshape(B, NS, NT).astype(np.float32),
            SL.reshape(B, NS, NT).astype(np.float32))


# revision 21
# speedup vs baseline: 2.2000x; 2.2000x over previous
"""DGMC-style graph matching network on 8 Trainium2 NeuronCores.

Reference math:
  psi(x) = relu(((I + A) @ x) @ W)   with A = dense ea-weighted adjacency
  h_s/h_t = psi(x_s/x_t, W1);  S_hat0[b] = h_s[b] @ h_t[b]^T
  10 steps: S = softmax(S_hat); r_t = S^T r_i; o_s/o_t = psi(r_i / r_t, W2)
            P_s = o_s@mw1 + mb1; P_t = o_t@mw1
            S_hat[s,t] += sum_c relu(P_s[s,c] - P_t[t,c]) * mw2[c]   (+mb2 dropped:
            a uniform logit shift cancels in every softmax downstream)
  outputs (softmax(S_hat0), softmax(S_hat_final))

Sharding: each of the 8 cores owns 256 consecutive global s-rows (2 cores
per graph). Edges are global (randint over all 2048 nodes), so o_t needs the
full r_t every step: each core computes its partial (S^T r) over its own
s-rows, a 64KB AllGather shares the 8 partials, and pair-sums rebuild r_t.
The adjacency (+I) is densified on the host; (I+A)@x runs as PE matmuls
against SBUF-resident A^T row-blocks. S_hat lives in PSUM for the whole
kernel and the relu'd pair-channel terms are reduced straight onto it with
tile_position matmuls (4 s-rows x 32 channels per 128-partition tile).
"""

import sys

import numpy as np

if "/opt/trn_rl_repo" not in sys.path:
    sys.path.insert(0, "/opt/trn_rl_repo")

B, NS, NT = 4, 512, 512
D_IN, C1, R, C2 = 128, 64, 32, 32
STEPS = 10
N = B * NS            # 2048 nodes per side
NCORES = 8
SROWS = N // NCORES   # 256 s-rows per core
KCH = N // 128        # 16 contraction chunks

_CACHE = {}


def _build_nc(steps=STEPS):
    import concourse.bacc as bacc
    import concourse.mybir as mybir
    import concourse.tile as tile

    DT = mybir.dt.float32
    AX = mybir.AxisListType
    OP = mybir.AluOpType
    AF = mybir.ActivationFunctionType

    nc = bacc.Bacc(None, target_bir_lowering=False, num_devices=NCORES)
    F32R = mybir.dt.float32r
    DTH = mybir.dt.float16

    def r_(ap):
        # walrus requires fp32r matmul operands to be *produced* rounded, so
        # step-chain tensors carry float32r dtype end-to-end; this helper only
        # reads fp32r bits back as plain fp32 for non-matmul consumers.
        return ap.bitcast(DT)

    AsT_d = nc.declare_dram_parameter("AsT", [KCH, 128, SROWS], F32R, isOutput=False)
    AtT_d = nc.declare_dram_parameter("AtT", [KCH, 128, NT], F32R, isOutput=False)
    xsch_d = nc.declare_dram_parameter("xsch", [KCH, 128, D_IN], DT, isOutput=False)
    xtch_d = nc.declare_dram_parameter("xtch", [KCH, 128, D_IN], DT, isOutput=False)
    rch_d = nc.declare_dram_parameter("rch", [steps, KCH, 128, R], F32R, isOutput=False)
    rown_d = nc.declare_dram_parameter("rown", [steps, 2, 128, R], DT, isOutput=False)
    w1_d = nc.declare_dram_parameter("w1", [D_IN, C1], DT, isOutput=False)
    w2_d = nc.declare_dram_parameter("w2", [R, C2], F32R, isOutput=False)
    mw1_d = nc.declare_dram_parameter("mw1", [C2, C2], F32R, isOutput=False)
    mb1_d = nc.declare_dram_parameter("mb1", [C2, 1], DT, isOutput=False)
    wblk_d = nc.declare_dram_parameter("wblk", [128, 8, 32], DTH, isOutput=False)
    mw1neg4_d = nc.declare_dram_parameter("mw1neg4", [C2, 128], F32R, isOutput=False)
    ident_d = nc.declare_dram_parameter("ident", [128, 128], DT, isOutput=False)
    s0_d = nc.declare_dram_parameter("S0", [SROWS, NT], DT, isOutput=True)
    sl_d = nc.declare_dram_parameter("SL", [SROWS, NT], DT, isOutput=True)

    with tile.TileContext(nc) as tc:
        with (
            tc.tile_pool(name="const", bufs=1) as cpool,
            tc.tile_pool(name="work", bufs=2) as wpool,
            tc.tile_pool(name="tg", bufs=8) as tgpool,
            tc.tile_pool(name="dram", bufs=2, space="DRAM") as dpool,
            tc.tile_pool(name="ps_shat", bufs=1, space="PSUM") as pshat,
            tc.tile_pool(name="ps_tmp", bufs=2, space="PSUM") as ptmp,
            tc.tile_pool(name="ps_tr", bufs=2, space="PSUM") as ptr,
            tc.tile_pool(name="ps_sch", bufs=2, space="PSUM") as psch,
        ):
            # ---- load constants ----
            a_sT = cpool.tile([128, KCH, SROWS], F32R)
            nc.sync.dma_start(a_sT[:], AsT_d.rearrange("j p s -> p j s"))
            a_tT = cpool.tile([128, KCH, NT], F32R)
            nc.sync.dma_start(a_tT[:], AtT_d.rearrange("j p s -> p j s"))
            xsch = cpool.tile([128, KCH, D_IN], DT)
            nc.sync.dma_start(xsch[:], xsch_d.rearrange("j p s -> p j s"))
            xtch = cpool.tile([128, KCH, D_IN], DT)
            nc.sync.dma_start(xtch[:], xtch_d.rearrange("j p s -> p j s"))
            rch = cpool.tile([128, steps, KCH, R], F32R)
            for i in range(steps):
                nc.sync.dma_start(rch[:, i, :, :], rch_d[i].rearrange("j p c -> p j c"))
            rown = cpool.tile([128, steps, 2, R], DT)
            for i in range(steps):
                nc.sync.dma_start(rown[:, i, :, :], rown_d[i].rearrange("q p c -> p q c"))
            w1 = cpool.tile([128, C1], DT)
            nc.sync.dma_start(w1[:], w1_d[:])
            w2 = cpool.tile([R, C2], F32R)
            nc.sync.dma_start(w2[:], w2_d[:])
            mw1 = cpool.tile([C2, C2], F32R)
            nc.sync.dma_start(mw1[:], mw1_d[:])
            mb1 = cpool.tile([C2, 1], DT)
            nc.sync.dma_start(mb1[:], mb1_d[:])
            wblk = cpool.tile([128, 8, 32], DTH)
            nc.sync.dma_start(wblk[:], wblk_d.rearrange("p v m -> p v m"))
            mw1neg4 = cpool.tile([C2, 128], F32R)
            nc.sync.dma_start(mw1neg4[:], mw1neg4_d[:])
            ident = cpool.tile([128, 128], DT)
            nc.sync.dma_start(ident[:], ident_d[:])

            # S_hat, PSUM-resident for the whole kernel (one bank per s-tile)
            shat = [pshat.tile([128, NT], DT, tag=f"shat{q}", name=f"shat{q}")
                    for q in range(2)]

            # ---- phase 1: h_s/h_t and S_hat0 ----
            usT_ps = ptmp.tile([128, SROWS], DT, tag="tmp", name="usT_ps")
            for j in range(KCH):
                nc.tensor.matmul(usT_ps[:], xsch[:, j, :], r_(a_sT[:, j, :]),
                                 start=(j == 0), stop=(j == KCH - 1))
            usT = wpool.tile([128, SROWS], DT, name="usT")
            nc.scalar.copy(usT[:], usT_ps[:])
            hsT_ps = ptmp.tile([C1, SROWS], DT, tag="tmp", name="hsT_ps")
            nc.tensor.matmul(hsT_ps[:], w1[:], usT[:], start=True, stop=True)
            hsT = wpool.tile([C1, SROWS], DT, name="hsT")
            nc.scalar.activation(hsT[:], hsT_ps[:], AF.Relu)

            utT_ps = ptmp.tile([128, NT], DT, tag="tmp", name="utT_ps")
            for j in range(KCH):
                nc.tensor.matmul(utT_ps[:], xtch[:, j, :], r_(a_tT[:, j, :]),
                                 start=(j == 0), stop=(j == KCH - 1))
            utT = wpool.tile([128, NT], DT, name="utT")
            nc.scalar.copy(utT[:], utT_ps[:])
            htT_ps = ptmp.tile([C1, NT], DT, tag="tmp", name="htT_ps")
            nc.tensor.matmul(htT_ps[:], w1[:], utT[:], start=True, stop=True)
            htT = wpool.tile([C1, NT], DT, name="htT")
            nc.scalar.activation(htT[:], htT_ps[:], AF.Relu)

            for q in range(2):
                nc.tensor.matmul(shat[q][:], hsT[:, q * 128:(q + 1) * 128], htT[:],
                                 start=True, stop=False, skip_group_check=True)

            # ---- step loop ----
            for i in range(steps):
                s_exp = []
                rinvs = []
                for q in range(2):
                    nmax = wpool.tile([128, 1], DT, tag="nmax", name="nmax")
                    nc.vector.tensor_reduce(nmax[:], shat[q][:], axis=AX.X,
                                            op=OP.max, negate=True)
                    se = wpool.tile([128, NT], F32R, tag="sexp", name="se")
                    rsum = wpool.tile([128, 1], DT, tag="rsum", name="rsum")
                    nc.scalar.activation(se[:], shat[q][:], AF.Exp,
                                         bias=nmax[:, 0:1], accum_out=rsum[:, 0:1])
                    rinv = wpool.tile([128, 1], DT, tag="rinv", name="rinv")
                    nc.vector.reciprocal(rinv[:], rsum[:])
                    s_exp.append(se)
                    rinvs.append(rinv)

                if i == 0:
                    for q in range(2):
                        s0st = wpool.tile([128, NT], DT, tag="sst", name="s0st")
                        nc.vector.tensor_scalar(s0st[:], r_(s_exp[q][:]),
                                                rinvs[q][:, 0:1], None, op0=OP.mult)
                        nc.sync.dma_start(s0_d[q * 128:(q + 1) * 128, :], s0st[:])

                # partial (S^T r) over own s-rows, computed directly in
                # [t, ch] wire layout: 8 small matmuls accumulate onto a
                # zeroed PSUM bank (all start=False, so slice order is free)
                pN_ps = ptr.tile([128, 128], DT, tag="tr", name="pN_ps")
                nc.vector.memset(pN_ps[:], 0.0)
                rscs = []
                for q in range(2):
                    rsc = wpool.tile([128, R], F32R, tag="rsc", name="rsc")
                    nc.vector.tensor_scalar(rsc[:], rown[:, i, q, :],
                                            rinvs[q][:, 0:1], None, op0=OP.mult)
                    rscs.append(rsc)
                for k in range(4):
                    for q in range(2):
                        nc.tensor.matmul(
                            pN_ps[:, k * 32:(k + 1) * 32],
                            s_exp[q][:, k * 128:(k + 1) * 128], rscs[q][:],
                            start=False, stop=(k == 3 and q == 1),
                            skip_group_check=True)

                pN_sb = wpool.tile([128, 128], DTH, name="pN_sb")
                nc.scalar.copy(pN_sb[:], pN_ps[:])
                cc_in = dpool.tile([NT, R], DTH, name="cc_in")
                nc.sync.dma_start(cc_in.rearrange("(k p) c -> p k c", p=128),
                                  pN_sb.rearrange("p (k c) -> p k c", k=4))
                cc_out = dpool.tile([NCORES, NT, R], DTH, name="cc_out")
                nc.gpsimd.collective_compute(
                    "AllGather", OP.bypass,
                    replica_groups=[list(range(NCORES))],
                    ins=[cc_in[:]], outs=[cc_out[:]],
                )

                # P_s chain for this step (independent of the collective;
                # fills the AllGather bubble): m_s = (I+A_s) r_i own rows
                ms_ps = psch.tile([R, SROWS], DT, tag="sch", name="ms_ps")
                for j in range(KCH):
                    nc.tensor.matmul(ms_ps[:], rch[:, i, j, :],
                                     a_sT[:, j, :],
                                     start=(j == 0), stop=(j == KCH - 1))
                ms_sb = wpool.tile([R, SROWS], F32R, name="ms_sb")
                nc.scalar.copy(ms_sb[:], ms_ps[:])
                os_ps = psch.tile([C2, SROWS], DT, tag="sch", name="os_ps")
                nc.tensor.matmul(os_ps[:], w2[:], ms_sb[:],
                                 start=True, stop=True)
                os_sb = wpool.tile([C2, SROWS], F32R, name="os_sb")
                nc.scalar.activation(os_sb[:], os_ps[:], AF.Relu)
                ps_ps = psch.tile([C2, SROWS], DT, tag="sch", name="ps_ps")
                nc.tensor.matmul(ps_ps[:], mw1[:], os_sb[:],
                                 start=True, stop=True)

                gath_ev = wpool.tile([128, KCH // 2, 2, R], DTH, name="gath_ev")
                gath_od = wpool.tile([128, KCH // 2, 2, R], DTH, name="gath_od")
                for g in range(B):
                    nc.sync.dma_start(
                        gath_ev[:, 2 * g:2 * g + 2, :, :],
                        cc_out[2 * g].rearrange("(k p) c -> p k c", p=128).rearrange("p (a b) c -> p a b c", a=2))
                    nc.sync.dma_start(
                        gath_od[:, 2 * g:2 * g + 2, :, :],
                        cc_out[2 * g + 1].rearrange("(k p) c -> p k c", p=128).rearrange("p (a b) c -> p a b c", a=2))
                rt = wpool.tile([128, KCH, R], F32R, name="rt")
                nc.vector.tensor_tensor(rt.rearrange("p j c -> p (j c)"),
                                        gath_ev.rearrange("p a b c -> p (a b c)"),
                                        gath_od.rearrange("p a b c -> p (a b c)"),
                                        op=OP.add)

                # u_t^T = ((I + A_t) @ r_t)^T for own graph's 512 t-rows
                ut_ps = ptmp.tile([R, NT], DT, tag="tmp", name="ut_ps")
                for j in range(KCH):
                    nc.tensor.matmul(ut_ps[:], rt[:, j, :], a_tT[:, j, :],
                                     start=(j == 0), stop=(j == KCH - 1))
                ut_sb = wpool.tile([R, NT], F32R, name="ut_sb")
                nc.scalar.copy(ut_sb[:], ut_ps[:])
                ot_ps = ptmp.tile([C2, NT], DT, tag="tmp", name="ot_ps")
                nc.tensor.matmul(ot_ps[:], w2[:], ut_sb[:],
                                 start=True, stop=True)
                ot_sb = wpool.tile([C2, NT], F32R, name="ot_sb")
                nc.scalar.activation(ot_sb[:], ot_ps[:], AF.Relu)
                # -P_t^T pre-replicated over the 4 s-slots of each partition
                # block, straight out of the PE: lhsT = -mw1 tiled 4x
                bneg_ps = ptmp.tile([128, NT], DT, tag="tmp", name="bneg_ps")
                nc.tensor.matmul(bneg_ps[:], mw1neg4[:], ot_sb[:],
                                 start=True, stop=True)
                bneg = wpool.tile([128, NT], DTH, name="bneg")
                nc.scalar.copy(bneg[:], bneg_ps[:])
                # Avec[32j+c, 32q+g] = P_s[s=128q+4g+j, c] + mb1[c]
                avec = wpool.tile([128, 64], DT, name="avec")
                for q in range(2):
                    view = ps_ps[:, q * 128:(q + 1) * 128].rearrange(
                        "p (g j) -> p g j", j=4)
                    for j in range(4):
                        nc.vector.tensor_scalar(avec[32 * j:32 * (j + 1),
                                                     q * 32:(q + 1) * 32],
                                                view[:, :, j], mb1[:, 0:1],
                                                None, op0=OP.add)

                # update: S_hat[q][4g:4g+4, :] += sum_c relu(P_s - P_t) * mw2[c]
                # rates ~ DVE 330ns, ACT 610ns, Pool 890ns per [128,512] tile:
                # split 8:5:3 per 16 and interleave so PE never starves on one
                # producer for long
                TG_ENG = {1: "A", 6: "A", 11: "A", 3: "P", 13: "P"}
                for q in range(2):
                    for g in range(32):
                        col = q * 32 + g
                        tg = tgpool.tile([128, NT], DTH, tag="tg", name="tg")
                        eng = TG_ENG.get(g % 16, "D")
                        if eng == "D":
                            nc.vector.tensor_scalar(tg[:], bneg[:],
                                                    avec[:, col:col + 1], 0.0,
                                                    op0=OP.add, op1=OP.max)
                        elif eng == "A":
                            nc.scalar.activation(tg[:], bneg[:], AF.Relu,
                                                 bias=avec[:, col:col + 1])
                        else:
                            nc.gpsimd.tensor_scalar(tg[:], bneg[:],
                                                    avec[:, col:col + 1], 0.0,
                                                    op0=OP.add, op1=OP.max)
                        k = g // 8
                        nc.tensor.matmul(shat[q][32 * k:32 * (k + 1), :],
                                         wblk[:, g % 8, :], tg[:],
                                         start=False, stop=False,
                                         skip_group_check=True,
                                         tile_position=(0, 32 * k))

            # ---- final softmax -> S_L ----
            for q in range(2):
                nmaxf = wpool.tile([128, 1], DT, tag="nmax", name="nmaxf")
                nc.vector.tensor_reduce(nmaxf[:], shat[q][:], axis=AX.X,
                                        op=OP.max, negate=True)
                sef = wpool.tile([128, NT], DT, tag="sexp", name="sef")
                rsumf = wpool.tile([128, 1], DT, tag="rsum", name="rsumf")
                nc.scalar.activation(sef[:], shat[q][:], AF.Exp,
                                     bias=nmaxf[:, 0:1], accum_out=rsumf[:, 0:1])
                rinvf = wpool.tile([128, 1], DT, tag="rinv", name="rinvf")
                nc.vector.reciprocal(rinvf[:], rsumf[:])
                slst = wpool.tile([128, NT], DT, tag="sst", name="slst")
                nc.vector.tensor_scalar(slst[:], sef[:], rinvf[:, 0:1],
                                        None, op0=OP.mult)
                nc.sync.dma_start(sl_d[q * 128:(q + 1) * 128, :], slst[:])

    nc.compile()
    return nc


def _host_prep(inputs, steps=STEPS):
    x_s = np.asarray(inputs["x_s"], np.float32)
    x_t = np.asarray(inputs["x_t"], np.float32)
    ei_s = np.asarray(inputs["edge_index_s"])
    ei_t = np.asarray(inputs["edge_index_t"])
    ea_s = np.asarray(inputs["edge_attr_s"], np.float32)
    ea_t = np.asarray(inputs["edge_attr_t"], np.float32)
    W1 = np.asarray(inputs["W1"], np.float32)
    W2 = np.asarray(inputs["W2"], np.float32)
    mw1 = np.asarray(inputs["mw1"], np.float32)
    mb1 = np.asarray(inputs["mb1"], np.float32)
    mw2 = np.asarray(inputs["mw2"], np.float32)
    r = np.asarray(inputs["r"], np.float32).reshape(-1, N, R)[:steps]

    A_s = np.zeros((N, N), np.float32)
    np.add.at(A_s, (ei_s[1], ei_s[0]), ea_s)
    A_s[np.arange(N), np.arange(N)] += 1.0
    A_t = np.zeros((N, N), np.float32)
    np.add.at(A_t, (ei_t[1], ei_t[0]), ea_t)
    A_t[np.arange(N), np.arange(N)] += 1.0

    rch = np.ascontiguousarray(r.reshape(steps, KCH, 128, R))
    xsch = np.ascontiguousarray(x_s.reshape(KCH, 128, D_IN))
    xtch = np.ascontiguousarray(x_t.reshape(KCH, 128, D_IN))
    wblk = np.zeros((128, 8, 32), np.float16)
    for v in range(8):
        for j in range(4):
            wblk[32 * j:32 * (j + 1), v, 4 * v + j] = mw2[:, 0].astype(np.float16)
    mw1neg4 = np.zeros((C2, 128), np.float32)
    for j in range(4):
        mw1neg4[:, 32 * j:32 * (j + 1)] = -mw1
    ident = np.eye(128, dtype=np.float32)
    mb1c = np.ascontiguousarray(mb1.reshape(C2, 1))

    in_maps = []
    for c in range(NCORES):
        rows = slice(SROWS * c, SROWS * (c + 1))
        trows = slice(NT * (c // 2), NT * (c // 2 + 1))
        AsT = np.ascontiguousarray(A_s[rows, :].T).reshape(KCH, 128, SROWS)
        AtT = np.ascontiguousarray(A_t[trows, :].T).reshape(KCH, 128, NT)
        rown = np.ascontiguousarray(
            r[:, SROWS * c:SROWS * (c + 1), :].reshape(steps, 2, 128, R))
        in_maps.append({
            "AsT": AsT, "AtT": AtT, "xsch": xsch, "xtch": xtch,
            "rch": rch, "rown": rown, "w1": W1, "w2": W2, "mw1": mw1,
            "mb1": mb1c, "wblk": wblk, "mw1neg4": mw1neg4, "ident": ident,
        })
    return in_maps


def kernel(**inputs):
    from concourse.bass_utils import run_bass_kernel_spmd

    if "nc" not in _CACHE:
        _CACHE["nc"] = _build_nc(STEPS)
    nc = _CACHE["nc"]

    in_maps = _host_prep(inputs, STEPS)
    res = run_bass_kernel_spmd(nc, in_maps, core_ids=list(range(NCORES)))
    outs = res.results
    S0 = np.concatenate([outs[c]["S0"] for c in range(NCORES)], axis=0)
    SL = np.concatenate([outs[c]["SL"] for c in range(NCORES)], axis=0)
    return (S0.reshape(B, NS, NT).astype(np.float32),
            SL.reshape(B, NS, NT).astype(np.float32))
